# revision 87
# baseline (speedup 1.0000x reference)
"""DLSMN scatter-memory + cache self-attention kernel for Trainium2.

Data-parallel over batch: batch b runs on NeuronCore b (8 cores), no
collectives.  Inside one core (one batch):

  phase A: per 128-token tile of y: PE-transpose y (f32r transposes) ->
           yT (bf16), bf16 matmuls (FWL weight loads) for write_vals /
           (logits,gate), gumbel-softmax routing (all Ln batched first ->
           2 ACT table loads total), weighted-scatter matmuls into 2 PSUM
           banks + a shared mass bank (3 banks total).
  phase B: slot update  upd = (1-g)*DECAY*old + g*updates/(mass+eps).
  phase C: PE-transpose cache2 -> cache2T (fp8e4).
  phase D: q/k/v projections with fp8 DoubleRow matmuls, bias folded
           into the PSUM evacuations: qT/kT bf16, v fp8e4.
  phase E: attention transposed, QK^T bf16.  Chunk structure: all 16 QK
           tiles first (exp on ScalarE streams into a persistent fp8 pT
           buffer), then the o-projection of the PREVIOUS chunk, then PV
           (fp8 DoubleRow) + denominator matmuls.  The previous chunk's
           softmax-normalization tail overlaps this chunk's QK segment.
  phase F: pipelined one chunk behind phase E: o-projection in fp8
           DoubleRow, residual + layernorm with a DVE-only Quake rsqrt,
           output DMA per n-tile.
"""

import numpy as np

import concourse.bacc as bacc
import concourse.mybir as mybir
import concourse.tile as tile
from concourse.bass_utils import run_bass_kernel_spmd
from concourse.masks import make_identity

F32 = mybir.dt.float32
F32R = mybir.dt.float32r
F16 = mybir.dt.float16
BF16 = mybir.dt.bfloat16
FP8 = mybir.dt.float8e4
I32 = mybir.dt.int32
AF = mybir.ActivationFunctionType
ALU = mybir.AluOpType
DR = mybir.MatmulPerfMode.DoubleRow

B = 8
S = 2048
D = 1024
DC = 512
K = 256
L = 8
H = 4
HD = 128
N = L * K
LAYER_IDX = 3
DECAY = 0.9
EPS = 1e-6
ST = S // 128   # 16 token tiles
NT = N // 128   # 16 slot tiles
DCH = D // 128  # 8 d_model chunks
CL = 256        # attention n-chunk length
NCH = N // CL   # 8 attention chunks
ATT_SCALE = float(1.0 / np.sqrt(np.float32(HD)))
QMAGIC = 0x5F3759DF + 1  # quake rsqrt magic (+1 for the xor-negate trick)

_INPUT_SPECS = {
    "y": (S, D), "cache": (N, DC), "gumbel_u": (S, K),
    "W_gate": (D, 1), "b_gate": (1,), "W_slot": (D, K), "b_slot": (K,),
    "gamma": (1,), "W_write": (D, DC), "b_write": (DC,),
    "Wq": (DC, DC), "bq": (DC,), "Wk": (DC, DC), "bk": (DC,),
    "Wv": (DC, DC), "bv": (DC,), "Wo": (DC, DC), "bo": (DC,),
    "ln_g": (DC,), "ln_b": (DC,),
}


def _build():
    nc = bacc.Bacc("TRN2", target_bir_lowering=False, debug=False, num_devices=B)

    a = {
        name: nc.dram_tensor(name, list(shape), F32, kind="ExternalInput").ap()
        for name, shape in _INPUT_SPECS.items()
    }
    out_dram = nc.dram_tensor("out", [N, DC], F32, kind="ExternalOutput").ap()

    y3 = a["y"].rearrange("(t p) d -> p t d", p=128)
    gum3 = a["gumbel_u"].rearrange("(t p) k -> p t k", p=128)
    cache3 = a["cache"].rearrange("(t p) d -> p t d", p=128)
    out3 = out_dram.rearrange("(t p) d -> p t d", p=128)

    with tile.TileContext(nc) as tc:
        with (
            tc.tile_pool(name="const", bufs=1) as const,
            tc.tile_pool(name="cachep", bufs=1) as cachep,
            tc.tile_pool(name="attn", bufs=1) as attn,
        ):
            # ---------------- constants ------------------------------------
            ident = const.tile([128, 128], F32)
            make_identity(nc, ident)
            ident_bf = const.tile([128, 128], BF16)
            nc.vector.tensor_copy(out=ident_bf, in_=ident)
            ident_r = const.tile([128, 128], F32R)
            nc.vector.tensor_copy(out=ident_r, in_=ident)
            ones_row_bf = const.tile([1, 128], BF16)
            nc.vector.memset(ones_row_bf, 1.0)
            ones_col2_bf = const.tile([128, 2], BF16)
            nc.vector.memset(ones_col2_bf, 1.0)
            ones8p = const.tile([128, 2, 16], FP8)
            nc.vector.memset(ones8p, 1.0)
            eps8_t = const.tile([128, 1], F32)
            nc.vector.memset(eps8_t, 1e-8)
            gamma_t = const.tile([128, 1], F32)
            nc.sync.dma_start(out=gamma_t, in_=a["gamma"].unsqueeze(0).to_broadcast([128, 1]))
            lng_bc = const.tile([128, DC], F32)
            nc.gpsimd.dma_start(out=lng_bc, in_=a["ln_g"].unsqueeze(0).to_broadcast([128, DC]))
            lnb_bc = const.tile([128, DC], F32)
            nc.gpsimd.dma_start(out=lnb_bc, in_=a["ln_b"].unsqueeze(0).to_broadcast([128, DC]))
            bwr_bc = const.tile([128, DC], F32)
            nc.gpsimd.dma_start(out=bwr_bc, in_=a["b_write"].unsqueeze(0).to_broadcast([128, DC]))
            # per-partition bias columns for q/k (out partition = dc within head)
            bq_col = const.tile([128, H], F32)
            nc.gpsimd.dma_start(out=bq_col, in_=a["bq"].rearrange("(h p) -> p h", p=128))
            bk_col = const.tile([128, H], F32)
            nc.gpsimd.dma_start(out=bk_col, in_=a["bk"].rearrange("(h p) -> p h", p=128))
            bsg_row = const.tile([1, K + 2], BF16)
            nc.gpsimd.dma_start(out=bsg_row[:, 0:K], in_=a["b_slot"].unsqueeze(0))
            nc.gpsimd.dma_start(out=bsg_row[:, K:K + 1], in_=a["b_gate"].unsqueeze(0))
            nc.gpsimd.dma_start(out=bsg_row[:, K + 1:K + 2], in_=a["b_gate"].unsqueeze(0))
            bor_row = const.tile([1, DC], BF16)
            nc.gpsimd.dma_start(out=bor_row, in_=a["bo"].unsqueeze(0))
            bvr_row = const.tile([1, DC], BF16)
            nc.gpsimd.dma_start(out=bvr_row, in_=a["bv"].unsqueeze(0))

            cache_sb = cachep.tile([128, NT, DC], BF16)
            cache_f = cache_sb

            # ---------------- persistent attention tiles -------------------
            c2t = attn.tile([128, 4, N], FP8)
            qT = attn.tile([128, H, N], BF16)
            kT = attn.tile([128, H, N], BF16)
            v_sb = attn.tile([128, NT, DC], FP8)
            wq8 = attn.tile([128, 4, DC], FP8)
            wk8 = attn.tile([128, 4, DC], FP8)
            wv8 = attn.tile([128, 4, DC], FP8)
            wo8 = attn.tile([128, 4, DC], FP8)
            aoT = attn.tile([128, H, N], FP8)
            pT = attn.tile([128, H, NT, CL], FP8)

            # ======================= phase A + B ===========================
            with (
                tc.tile_pool(name="wA", bufs=1) as wA,
                tc.tile_pool(name="pA", bufs=2) as pA,
                tc.tile_pool(name="pAs", bufs=3) as pAs,
                tc.tile_pool(name="psT", bufs=2, space="PSUM") as psT,
                tc.tile_pool(name="psWV", bufs=2, space="PSUM") as psWV,
                tc.tile_pool(name="psLG", bufs=1, space="PSUM") as psLG,
                tc.tile_pool(name="psU", bufs=1, space="PSUM") as psU,
            ):
                # A-phase weights: fast f32 DMA to staging on the vector
                # queue, chunked DVE casts -> bf16 (the gpsimd casting-DMA
                # path is only ~130GB/s and would gate phases A and CD)
                wwr = wA.tile([128, DCH, DC], BF16)
                wsg = wA.tile([128, DCH, K + 2], BF16)
                wwr3 = a["W_write"].rearrange("(c p) d -> p c d", p=128)
                wsl3 = a["W_slot"].rearrange("(c p) k -> p c k", p=128)
                for cc in range(DCH):
                    nc.gpsimd.dma_start(out=wwr[:, cc, :], in_=wwr3[:, cc, :])
                    nc.gpsimd.dma_start(out=wsg[:, cc, 0:K], in_=wsl3[:, cc, :])
                nc.gpsimd.dma_start(out=wsg[:, :, K:K + 1], in_=a["W_gate"].rearrange("(c p) o -> p c o", p=128))
                nc.gpsimd.dma_start(out=wsg[:, :, K + 1:K + 2], in_=a["W_gate"].rearrange("(c p) o -> p c o", p=128))
                # cache for phases B/C/F (casting DMA fp32 -> bf16)
                nc.gpsimd.dma_start(out=cache_sb, in_=cache3)
                # attention weights (fp8 casting DMAs)
                wq3 = a["Wq"].rearrange("(c p) d -> p c d", p=128)
                wk3 = a["Wk"].rearrange("(c p) d -> p c d", p=128)
                wv3 = a["Wv"].rearrange("(c p) d -> p c d", p=128)
                wo3 = a["Wo"].rearrange("(c p) d -> p c d", p=128)
                nc.gpsimd.dma_start(out=wq8, in_=wq3)
                nc.gpsimd.dma_start(out=wk8, in_=wk3)
                nc.gpsimd.dma_start(out=wv8, in_=wv3)
                nc.gpsimd.dma_start(out=wo8, in_=wo3)

                # prologue: first two y tiles DMA'd + cast (DVE) before the
                # Ln prepass so the PE transposes start immediately
                y_bf_pre = {}
                for i in range(2):
                    y_t = pA.tile([128, D], F32, tag="y", name=f"ypre{i}")
                    nc.sync.dma_start(out=y_t, in_=y3[:, i, :])
                    y_bf = pA.tile([128, D], BF16, tag="ybf", name=f"ybfpre{i}")
                    nc.vector.tensor_copy(out=y_bf, in_=y_t)
                    y_bf_pre[i] = y_bf

                # gumbel Ln prepass: all Ln ops batched (one ACT table set)
                lnz_all = wA.tile([128, ST, K], F16)
                for g in range(4):
                    gum = pA.tile([128, 4, K], F32, tag="gum")
                    nc.sync.dma_start(out=gum, in_=gum3[:, 4 * g:4 * g + 4, :])
                    lnu = pA.tile([128, 4, K], F32, tag="lnu")
                    nc.scalar.activation(lnu, gum, AF.Ln, bias=eps8_t)
                    nc.scalar.activation(lnz_all[:, 4 * g:4 * g + 4, :], lnu, AF.Ln,
                                         bias=eps8_t, scale=-1.0)

                # persistent scatter accumulators:
                #   updates: one bank per kc (512 fp32)
                #   mass: single shared bank [128, 4] (cols 2kc:2kc+2)
                ps_upd = [psU.tile([128, DC], F32, name=f"upd{kc}", tag=f"upd{kc}")
                          for kc in range(2)]
                ps_mass = psU.tile([128, 4], F32, name="mass", tag="mass")

                prev = [None]

                def flush_scatter():
                    if prev[0] is None:
                        return
                    j, w_j, wv_j = prev[0]
                    for kc in range(2):
                        lhs = w_j[:, kc * 128:(kc + 1) * 128]
                        nc.tensor.matmul(ps_upd[kc], lhs, wv_j,
                                         start=(j == 0), stop=(j == ST - 1))
                        nc.tensor.matmul(ps_mass[:, 2 * kc:2 * kc + 2], lhs,
                                         ones_col2_bf,
                                         start=(j == 0 and kc == 0),
                                         stop=(j == ST - 1 and kc == 1),
                                         skip_group_check=True)
                    prev[0] = None

                for i in range(ST):
                    if i in y_bf_pre:
                        y_bf = y_bf_pre[i]
                    else:
                        y_t = pA.tile([128, D], F32, tag="y")
                        nc.sync.dma_start(out=y_t, in_=y3[:, i, :])
                        y_bf = pA.tile([128, D], BF16, tag="ybf")
                        nc.scalar.copy(out=y_bf, in_=y_t)

                    # transpose y tile -> yT (bf16 transposes, cheap LDW)
                    yT = pA.tile([128, D], BF16, tag="yT")
                    for g in range(2):
                        tr = psT.tile([128, 512], BF16, tag="tr")
                        for cc in range(4):
                            c = 4 * g + cc
                            nc.tensor.transpose(
                                tr[:, cc * 128:(cc + 1) * 128],
                                y_bf[:, c * 128:(c + 1) * 128],
                                ident_bf,
                            )
                        nc.vector.tensor_copy(out=yT[:, g * 512:(g + 1) * 512],
                                              in_=tr)

                    # write_vals / (logits, gate) matmuls
                    ps_wv = psWV.tile([128, DC], F32, tag="wv")
                    for c in range(DCH):
                        nc.tensor.matmul(
                            ps_wv, yT[:, c * 128:(c + 1) * 128], wwr[:, c, :],
                            start=(c == 0), stop=(c == DCH - 1),
                        )
                    ps_lg = psLG.tile([128, K + 2], F32, tag="lg")
                    for c in range(DCH):
                        nc.tensor.matmul(
                            ps_lg, yT[:, c * 128:(c + 1) * 128], wsg[:, c, :],
                            start=(c == 0), stop=False,
                        )
                    nc.tensor.matmul(ps_lg, ones_row_bf, bsg_row,
                                     start=False, stop=True)

                    # scatter matmuls for the previous tile (keeps PE dense
                    # while this tile's DVE/ACT chain runs)
                    flush_scatter()

                    # t = gamma*logits - lnz
                    t_sb = pAs.tile([128, K], F32, tag="tsb")
                    nc.vector.scalar_tensor_tensor(
                        out=t_sb, in0=ps_lg[:, 0:K], scalar=gamma_t,
                        in1=lnz_all[:, i, :], op0=ALU.mult, op1=ALU.subtract,
                    )
                    # scores = sigmoid(gate)
                    sc_e = pAs.tile([128, 1], F32, tag="sce")
                    nc.scalar.activation(sc_e, ps_lg[:, K:K + 1], AF.Exp, scale=-1.0)
                    sc1 = pAs.tile([128, 1], F32, tag="sc1")
                    nc.vector.tensor_scalar_add(sc1, sc_e, 1.0)
                    scores = pAs.tile([128, 1], F32, tag="scores")
                    nc.vector.reciprocal(scores, sc1)
                    # p_unnorm = exp(t) with fused row-sum
                    p_un = pAs.tile([128, K], F32, tag="pun")
                    rs = pAs.tile([128, 1], F32, tag="rs")
                    nc.scalar.activation(p_un, t_sb, AF.Exp, accum_out=rs)
                    rrs = pAs.tile([128, 1], F32, tag="rrs")
                    nc.vector.reciprocal(rrs, rs)
                    s2 = pAs.tile([128, 1], F32, tag="s2")
                    nc.vector.tensor_tensor(s2, scores, rrs, ALU.mult)
                    w_sb = pAs.tile([128, K], BF16, tag="wsb")
                    nc.vector.tensor_scalar_mul(w_sb, p_un, s2)
                    # wv_sb = write_vals + b_write
                    wv_sb = pAs.tile([128, DC], BF16, tag="wvsb")
                    nc.vector.scalar_tensor_tensor(
                        out=wv_sb, in0=ps_wv, scalar=1.0,
                        in1=bwr_bc, op0=ALU.mult, op1=ALU.add,
                    )
                    prev[0] = (i, w_sb, wv_sb)

                flush_scatter()

                # ------- phase B: slot update, overwrite cache rows -------
                base_t = LAYER_IDX * K // 128  # n-tile 6
                for kc in range(2):
                    mass = pAs.tile([128, 1], F32, tag="mass")
                    nc.vector.tensor_copy(out=mass, in_=ps_mass[:, 2 * kc:2 * kc + 1])
                    m1 = pAs.tile([128, 1], F32, tag="m1")
                    nc.vector.tensor_scalar_add(m1, mass, EPS)
                    rm = pAs.tile([128, 1], F32, tag="rm")
                    nc.vector.reciprocal(rm, m1)
                    m2 = pAs.tile([128, 1], F32, tag="m2")
                    nc.vector.tensor_scalar_add(m2, mass, 1.0)
                    rg = pAs.tile([128, 1], F32, tag="rg")
                    nc.vector.reciprocal(rg, m2)
                    g_t = pAs.tile([128, 1], F32, tag="gt")
                    nc.vector.tensor_tensor(g_t, mass, rg, ALU.mult)
                    co = pAs.tile([128, 1], F32, tag="co")
                    nc.vector.tensor_scalar(co, g_t, -DECAY, DECAY, ALU.mult, ALU.add)
                    cn = pAs.tile([128, 1], F32, tag="cn")
                    nc.vector.tensor_tensor(cn, g_t, rm, ALU.mult)

                    told = pA.tile([128, DC], F32, tag="told")
                    nc.vector.tensor_scalar_mul(told, cache_f[:, base_t + kc, :], co)
                    nc.vector.scalar_tensor_tensor(
                        out=cache_sb[:, base_t + kc, :],
                        in0=ps_upd[kc], scalar=cn, in1=told,
                        op0=ALU.mult, op1=ALU.add,
                    )

            # ============== phases C + D + E + F (shared PSUM) =============
            # one PSUM layout: 2x [128,1024] shared slots (C-transposes,
            # D-projections, E attention scores, F o-proj) + ao 2 + den 1
            # + 1 spare slot bank, so the scheduler can overlap phases.
            with (
                tc.tile_pool(name="psS", bufs=2, space="PSUM") as psS,
                tc.tile_pool(name="pEs", bufs=2) as pEs,
                tc.tile_pool(name="pF", bufs=2) as pF,
                tc.tile_pool(name="psX", bufs=2, space="PSUM") as psX,
                tc.tile_pool(name="psAo", bufs=1, space="PSUM") as psAo,
            ):
                # ------- phase C: cache2 -> cache2T (fp8) ------------------
                evac_flip = [0]

                def evac_copy(out_ap, in_ap):
                    if evac_flip[0] % 2 == 0:
                        nc.scalar.copy(out=out_ap, in_=in_ap)
                    else:
                        nc.vector.tensor_copy(out=out_ap, in_=in_ap)
                    evac_flip[0] += 1

                def d_slot(idx, name):
                    if idx % 2 == 0:
                        return psS.tile([128, 512], F32, tag="att", name=name)
                    return psX.tile([128, 512], F32, tag="x", name=name)

                slot_i = [0]
                for j in range(4):
                    for tg in range(4):
                        ps = (psS.tile([128, 512], BF16, tag="att", name=f"ctr{j}_{tg}")
                              if tg % 2 == 0 else
                              psX.tile([128, 512], BF16, tag="x", name=f"ctr{j}_{tg}"))
                        for tt in range(4):
                            t = tg * 4 + tt
                            nc.tensor.transpose(
                                ps[:, tt * 128:(tt + 1) * 128],
                                cache_sb[:, t, j * 128:(j + 1) * 128],
                                ident_bf,
                            )
                        evac_copy(c2t[:, j, tg * 512:(tg + 1) * 512], ps)

                # ------- phase D: projections (fp8 DoubleRow) --------------
                # v first (needed early in E), then k/q per n-chunk so the
                # attention exp stream can begin before q/k fully finish
                for m in range(NT):
                    ps = d_slot(slot_i[0], f"v{m}"); slot_i[0] += 1
                    for g in range(2):
                        nc.tensor.matmul(
                            ps, c2t[:, 2 * g:2 * g + 2, m * 128:(m + 1) * 128],
                            wv8[:, 2 * g:2 * g + 2, :],
                            start=(g == 0), stop=False, perf_mode=DR,
                        )
                    nc.tensor.matmul(ps, ones_row_bf, bvr_row,
                                     start=False, stop=True)
                    evac_copy(v_sb[:, m, :], ps)
                for c in range(4):
                    for dst, w8, b_col in ((kT, wk8, bk_col), (qT, wq8, bq_col)):
                        for h in range(H):
                            ps = d_slot(slot_i[0], f"qk{c}_{h}_{0 if dst is kT else 1}")
                            slot_i[0] += 1
                            for g in range(2):
                                nc.tensor.matmul(
                                    ps, w8[:, 2 * g:2 * g + 2, h * 128:(h + 1) * 128],
                                    c2t[:, 2 * g:2 * g + 2, c * 512:(c + 1) * 512],
                                    start=(g == 0), stop=(g == 1), perf_mode=DR,
                                )
                            if evac_flip[0] % 2 == 0:
                                nc.scalar.activation(
                                    dst[:, h, c * 512:(c + 1) * 512], ps,
                                    AF.Identity, bias=b_col[:, h:h + 1])
                            else:
                                nc.vector.tensor_scalar_add(
                                    dst[:, h, c * 512:(c + 1) * 512], ps,
                                    b_col[:, h:h + 1])
                            evac_flip[0] += 1
                def emit_F_half(c, half, st):
                    # phase F o-proj + LN stats for n-tile 2c+half
                    if half == 0:
                        st["mean2"] = pF.tile([128, 2], F32, tag="mean2",
                                              name=f"mean2_{c}")
                        st["var2"] = pF.tile([128, 2], F32, tag="var2",
                                             name=f"var2_{c}")
                        st["r"] = []
                    mean2, var2 = st["mean2"], st["var2"]
                    t = 2 * c + half
                    ps_o = psS.tile([128, DC], F32, tag="att",
                                    name=f"o{c}_{half}")
                    for g in range(2):
                        nc.tensor.matmul(
                            ps_o, aoT[:, 2 * g:2 * g + 2, t * 128:(t + 1) * 128],
                            wo8[:, 2 * g:2 * g + 2, :],
                            start=(g == 0), stop=False, perf_mode=DR,
                        )
                    nc.tensor.matmul(ps_o, ones_row_bf, bor_row,
                                     start=False, stop=True)
                    r_t = pF.tile([128, DC], F32, tag="r")
                    rsum = pF.tile([128, 1], F32, tag="rsum")
                    nc.vector.scalar_tensor_tensor(
                        out=r_t, in0=ps_o, scalar=1.0,
                        in1=cache_f[:, t, :],
                        op0=ALU.mult, op1=ALU.add, accum_out=rsum,
                    )
                    nc.vector.tensor_scalar_mul(
                        mean2[:, half:half + 1], rsum, 1.0 / DC)
                    scratch = pF.tile([128, DC], F32, tag="scratch")
                    nc.vector.scalar_tensor_tensor(
                        out=scratch, in0=r_t,
                        scalar=mean2[:, half:half + 1], in1=r_t,
                        op0=ALU.subtract, op1=ALU.mult,
                        accum_out=var2[:, half:half + 1],
                    )
                    st["r"].append(r_t)

                def emit_F_tail(c, st):
                    mean2, var2, r_ts = st["mean2"], st["var2"], st["r"]
                    # rstd = 1/sqrt(var/DC + 1e-5), quake + 2 Newton (DVE only)
                    nc.vector.tensor_scalar(var2, var2, 1.0 / DC, 1e-5,
                                            ALU.mult, ALU.add)
                    vh = pF.tile([128, 2], F32, tag="vh")
                    nc.vector.tensor_scalar_mul(vh, var2, -0.5)
                    qi = pF.tile([128, 2], I32, tag="qi")
                    nc.vector.tensor_scalar(qi, var2.bitcast(I32), 1, -1,
                                            ALU.logical_shift_right, ALU.bitwise_xor)
                    rstd = pF.tile([128, 2], F32, tag="rstd")
                    nc.vector.tensor_scalar_add(rstd.bitcast(I32), qi, QMAGIC)
                    yy = pF.tile([128, 2], F32, tag="yy")
                    for _ in range(2):
                        nc.vector.tensor_tensor(yy, rstd, rstd, ALU.mult)
                        nc.vector.tensor_tensor(yy, yy, vh, ALU.mult)
                        nc.vector.tensor_scalar_add(yy, yy, 1.5)
                        nc.vector.tensor_tensor(rstd, rstd, yy, ALU.mult)
                    for half in range(2):
                        t = 2 * c + half
                        t1 = pF.tile([128, DC], F32, tag="t1")
                        nc.vector.tensor_scalar(
                            t1, r_ts[half], mean2[:, half:half + 1],
                            rstd[:, half:half + 1], ALU.subtract, ALU.mult)
                        t2 = pF.tile([128, DC], F32, tag="t2")
                        nc.vector.scalar_tensor_tensor(
                            out=t2, in0=t1, scalar=1.0, in1=lng_bc,
                            op0=ALU.mult, op1=ALU.mult,
                        )
                        o_sb = pF.tile([128, DC], F32, tag="osb")
                        nc.vector.scalar_tensor_tensor(
                            out=o_sb, in0=t2, scalar=1.0, in1=lnb_bc,
                            op0=ALU.mult, op1=ALU.add,
                        )
                        nc.sync.dma_start(out=out3[:, t, :], in_=o_sb)

                f_state = {}
                for c in range(NCH):
                    # QK/exp stream with PV (fp8 DoubleRow) + denominator
                    # matmuls interleaved once the previous chunk's ao banks
                    # have drained (pairs 0..6 after QK m=9..15, pair 7 last)
                    ps_aoA = psAo.tile([128, 512], F32, name=f"aoA{c}", tag="aoA")
                    ps_aoB = psAo.tile([128, 512], F32, name=f"aoB{c}", tag="aoB")
                    ps_denA = psX.tile([128, 512], F32, name=f"denA{c}", tag="x")
                    ps_denB = psX.tile([128, 512], F32, name=f"denB{c}", tag="x")

                    def emit_pair(j):
                        for h in range(H):
                            ps_ao = ps_aoA if h < 2 else ps_aoB
                            nc.tensor.matmul(
                                ps_ao[:, (h % 2) * CL:(h % 2 + 1) * CL],
                                v_sb[:, 2 * j:2 * j + 2, h * 128:(h + 1) * 128],
                                pT[:, h, 2 * j:2 * j + 2, :],
                                start=(j == 0 and h % 2 == 0), stop=False,
                                perf_mode=DR,
                            )
                        # denominators: DoubleRow column-sums, h-pair per bank
                        for h in range(H):
                            ps_den = ps_denA if h < 2 else ps_denB
                            nc.tensor.matmul(
                                ps_den[0:1, (h % 2) * CL:(h % 2 + 1) * CL],
                                ones8p[:, :, 0:1],
                                pT[:, h, 2 * j:2 * j + 2, :],
                                start=(j == 0), stop=False,
                                perf_mode=DR,
                                skip_group_check=True,
                            )

                    def emit_single(m2):
                        # last two m-tiles as plain fp8 matmuls so m14's work
                        # overlaps exp(m15) and the boundary tail stays short
                        last = (m2 == NT - 1)
                        for h in range(H):
                            ps_ao = ps_aoA if h < 2 else ps_aoB
                            nc.tensor.matmul(
                                ps_ao[:, (h % 2) * CL:(h % 2 + 1) * CL],
                                v_sb[:, m2, h * 128:(h + 1) * 128],
                                pT[:, h, m2, :],
                                start=False, stop=(last and h % 2 == 1),
                            )
                        for h in range(H):
                            ps_den = ps_denA if h < 2 else ps_denB
                            nc.tensor.matmul(
                                ps_den[0:1, (h % 2) * CL:(h % 2 + 1) * CL],
                                ones8p[:, 0, 0:1],
                                pT[:, h, m2, :],
                                start=False, stop=last,
                                skip_group_check=True,
                            )

                    for m in range(NT):
                        ps_a = psS.tile([128, H * CL], F32, tag="att")
                        for h in range(H):
                            nc.tensor.matmul(
                                ps_a[:, h * CL:(h + 1) * CL],
                                kT[:, h, m * 128:(m + 1) * 128],
                                qT[:, h, c * CL:(c + 1) * CL],
                                start=True, stop=True,
                            )
                        nc.scalar.activation(
                            pT[:, :, m, :],
                            ps_a.rearrange("p (h n) -> p h n", h=H),
                            AF.Exp, scale=ATT_SCALE)
                        if c > 0:
                            if m == 7:
                                emit_F_half(c - 1, 0, f_state)
                            elif m == 8:
                                emit_F_half(c - 1, 1, f_state)
                                emit_F_tail(c - 1, f_state)
                        if m >= 9 and m <= 15:
                            emit_pair(m - 9)
                        if m == 15:
                            emit_single(14)
                    emit_single(15)

                    # 1/den via one Newton step from y0=2^-11 (den ~ 2048),
                    # broadcast den rows, fused normalize evac
                    den_row = pEs.tile([1, H * CL], F32, tag="denrow")
                    nc.vector.tensor_copy(out=den_row[:, 0:512], in_=ps_denA[0:1, :])
                    nc.vector.tensor_copy(out=den_row[:, 512:1024], in_=ps_denB[0:1, :])
                    bc_den = pEs.tile([128, H * CL], F32, tag="bcden")
                    nc.gpsimd.partition_broadcast(bc_den, den_row)
                    # y1 = y0*(2 - den*y0),  y0 = 2^-11
                    Y0 = 1.0 / 2048.0
                    t2_nr = pEs.tile([128, H * CL], F32, tag="t2nr")
                    nc.vector.tensor_scalar(t2_nr, bc_den, -Y0, 2.0,
                                            ALU.mult, ALU.add)
                    rden_bc = pEs.tile([128, H * CL], F32, tag="rdenbc")
                    nc.vector.tensor_scalar_mul(rden_bc, t2_nr, Y0)
                    for h in range(H):
                        ps_ao = ps_aoA if h < 2 else ps_aoB
                        nc.vector.scalar_tensor_tensor(
                            out=aoT[:, h, c * CL:(c + 1) * CL],
                            in0=ps_ao[:, (h % 2) * CL:(h % 2 + 1) * CL],
                            scalar=1.0, in1=rden_bc[:, h * CL:(h + 1) * CL],
                            op0=ALU.mult, op1=ALU.mult,
                        )

                emit_F_half(NCH - 1, 0, f_state)
                emit_F_half(NCH - 1, 1, f_state)
                emit_F_tail(NCH - 1, f_state)

    nc.compile()
    return nc


_NC_CACHE = {}


def _get_nc():
    if "nc" not in _NC_CACHE:
        _NC_CACHE["nc"] = _build()
    return _NC_CACHE["nc"]


def _in_maps(inputs):
    per_batch = {"y", "cache", "gumbel_u"}
    maps = []
    for b in range(B):
        m = {}
        for name in _INPUT_SPECS:
            arr = np.ascontiguousarray(np.asarray(inputs[name], dtype=np.float32))
            m[name] = arr[b] if name in per_batch else arr
        maps.append(m)
    return maps


def _execute(inputs, trace=False):
    nc = _get_nc()
    res = run_bass_kernel_spmd(nc, _in_maps(inputs), list(range(B)), trace=trace)
    out = np.stack([res.results[b]["out"] for b in range(B)]).astype(np.float32)
    return out, res


def kernel(**inputs) -> np.ndarray:
    out, _ = _execute(inputs)
    return out


# revision 88
# speedup vs baseline: 1.1682x; 1.1682x over previous
"""DLSMN scatter-memory + cache self-attention kernel for Trainium2.

Data-parallel over batch: batch b runs on NeuronCore b (8 cores), no
collectives.  Inside one core (one batch):

  phase A: per 128-token tile of y: PE-transpose y (f32r transposes) ->
           yT (bf16), bf16 matmuls (FWL weight loads) for write_vals /
           (logits,gate), gumbel-softmax routing (all Ln batched first ->
           2 ACT table loads total), weighted-scatter matmuls into 2 PSUM
           banks + a shared mass bank (3 banks total).
  phase B: slot update  upd = (1-g)*DECAY*old + g*updates/(mass+eps).
  phase C: PE-transpose cache2 -> cache2T (fp8e4).
  phase D: q/k/v projections with fp8 DoubleRow matmuls, bias folded
           into the PSUM evacuations: qT/kT bf16, v fp8e4.
  phase E: attention transposed, QK^T bf16.  Chunk structure: all 16 QK
           tiles first (exp on ScalarE streams into a persistent fp8 pT
           buffer), then the o-projection of the PREVIOUS chunk, then PV
           (fp8 DoubleRow) + denominator matmuls.  The previous chunk's
           softmax-normalization tail overlaps this chunk's QK segment.
  phase F: pipelined one chunk behind phase E: o-projection in fp8
           DoubleRow, residual + layernorm with a DVE-only Quake rsqrt,
           output DMA per n-tile.
"""

import numpy as np

import concourse.bacc as bacc
import concourse.mybir as mybir
import concourse.tile as tile
from concourse.bass_utils import run_bass_kernel_spmd
from concourse.masks import make_identity

F32 = mybir.dt.float32
F32R = mybir.dt.float32r
F16 = mybir.dt.float16
BF16 = mybir.dt.bfloat16
FP8 = mybir.dt.float8e4
I32 = mybir.dt.int32
AF = mybir.ActivationFunctionType
ALU = mybir.AluOpType
DR = mybir.MatmulPerfMode.DoubleRow

B = 8
S = 2048
D = 1024
DC = 512
K = 256
L = 8
H = 4
HD = 128
N = L * K
LAYER_IDX = 3
DECAY = 0.9
EPS = 1e-6
ST = S // 128   # 16 token tiles
NT = N // 128   # 16 slot tiles
DCH = D // 128  # 8 d_model chunks
CL = 256        # attention n-chunk length
NCH = N // CL   # 8 attention chunks
ATT_SCALE = float(1.0 / np.sqrt(np.float32(HD)))
QMAGIC = 0x5F3759DF + 1  # quake rsqrt magic (+1 for the xor-negate trick)

_INPUT_SPECS = {
    "y": (S, D), "cache": (N, DC), "gumbel_u": (S, K),
    "W_gate": (D, 1), "b_gate": (1,), "W_slot": (D, K), "b_slot": (K,),
    "gamma": (1,), "W_write": (D, DC), "b_write": (DC,),
    "Wq": (DC, DC), "bq": (DC,), "Wk": (DC, DC), "bk": (DC,),
    "Wv": (DC, DC), "bv": (DC,), "Wo": (DC, DC), "bo": (DC,),
    "ln_g": (DC,), "ln_b": (DC,),
}


def _build():
    nc = bacc.Bacc("TRN2", target_bir_lowering=False, debug=False, num_devices=B)

    a = {
        name: nc.dram_tensor(name, list(shape), F32, kind="ExternalInput").ap()
        for name, shape in _INPUT_SPECS.items()
    }
    out_dram = nc.dram_tensor("out", [N, DC], F32, kind="ExternalOutput").ap()

    y3 = a["y"].rearrange("(t p) d -> p t d", p=128)
    gum3 = a["gumbel_u"].rearrange("(t p) k -> p t k", p=128)
    cache3 = a["cache"].rearrange("(t p) d -> p t d", p=128)
    out3 = out_dram.rearrange("(t p) d -> p t d", p=128)

    with tile.TileContext(nc) as tc:
        with (
            tc.tile_pool(name="const", bufs=1) as const,
            tc.tile_pool(name="cachep", bufs=1) as cachep,
            tc.tile_pool(name="attn", bufs=1) as attn,
        ):
            # ---------------- constants ------------------------------------
            ident = const.tile([128, 128], F32)
            make_identity(nc, ident)
            ident_bf = const.tile([128, 128], BF16)
            nc.vector.tensor_copy(out=ident_bf, in_=ident)
            ident_r = const.tile([128, 128], F32R)
            nc.vector.tensor_copy(out=ident_r, in_=ident)
            ones_row_bf = const.tile([1, 128], BF16)
            nc.vector.memset(ones_row_bf, 1.0)
            ones_col2_bf = const.tile([128, 2], BF16)
            nc.vector.memset(ones_col2_bf, 1.0)
            ones8p = const.tile([128, 2, 16], FP8)
            nc.vector.memset(ones8p, 1.0)
            eps8_t = const.tile([128, 1], F32)
            nc.vector.memset(eps8_t, 1e-8)
            gamma_t = const.tile([128, 1], F32)
            nc.sync.dma_start(out=gamma_t, in_=a["gamma"].unsqueeze(0).to_broadcast([128, 1]))
            lng_bc = const.tile([128, DC], F32)
            nc.gpsimd.dma_start(out=lng_bc, in_=a["ln_g"].unsqueeze(0).to_broadcast([128, DC]))
            lnb_bc = const.tile([128, DC], F32)
            nc.gpsimd.dma_start(out=lnb_bc, in_=a["ln_b"].unsqueeze(0).to_broadcast([128, DC]))
            bwr_bc = const.tile([128, DC], F32)
            nc.gpsimd.dma_start(out=bwr_bc, in_=a["b_write"].unsqueeze(0).to_broadcast([128, DC]))
            # per-partition bias columns for q/k (out partition = dc within head)
            bq_col = const.tile([128, H], F32)
            nc.gpsimd.dma_start(out=bq_col, in_=a["bq"].rearrange("(h p) -> p h", p=128))
            bk_col = const.tile([128, H], F32)
            nc.gpsimd.dma_start(out=bk_col, in_=a["bk"].rearrange("(h p) -> p h", p=128))
            bsg_row = const.tile([1, K + 2], BF16)
            nc.gpsimd.dma_start(out=bsg_row[:, 0:K], in_=a["b_slot"].unsqueeze(0))
            nc.gpsimd.dma_start(out=bsg_row[:, K:K + 1], in_=a["b_gate"].unsqueeze(0))
            nc.gpsimd.dma_start(out=bsg_row[:, K + 1:K + 2], in_=a["b_gate"].unsqueeze(0))
            bor_row = const.tile([1, DC], BF16)
            nc.gpsimd.dma_start(out=bor_row, in_=a["bo"].unsqueeze(0))
            bvr_row = const.tile([1, DC], BF16)
            nc.gpsimd.dma_start(out=bvr_row, in_=a["bv"].unsqueeze(0))

            cache_sb = cachep.tile([128, NT, DC], BF16)
            cache_f = cache_sb

            # ---------------- persistent attention tiles -------------------
            c2t = attn.tile([128, 4, N], FP8)
            qT = attn.tile([128, H, N], BF16)
            kT = attn.tile([128, H, N], BF16)
            v_sb = attn.tile([128, NT, DC], FP8)
            wq8 = attn.tile([128, 4, DC], FP8)
            wk8 = attn.tile([128, 4, DC], FP8)
            wv8 = attn.tile([128, 4, DC], FP8)
            wo8 = attn.tile([128, 4, DC], FP8)
            aoT = attn.tile([128, H, N], FP8)
            pT = attn.tile([128, H, NT, CL], FP8)

            # ======================= phase A + B ===========================
            with (
                tc.tile_pool(name="wA", bufs=1) as wA,
                tc.tile_pool(name="pA", bufs=2) as pA,
                tc.tile_pool(name="pAs", bufs=3) as pAs,
                tc.tile_pool(name="psT", bufs=2, space="PSUM") as psT,
                tc.tile_pool(name="psWV", bufs=2, space="PSUM") as psWV,
                tc.tile_pool(name="psLG", bufs=1, space="PSUM") as psLG,
                tc.tile_pool(name="psU", bufs=1, space="PSUM") as psU,
            ):
                # A-phase weights: fast f32 DMA to staging on the vector
                # queue, chunked DVE casts -> bf16 (the gpsimd casting-DMA
                # path is only ~130GB/s and would gate phases A and CD)
                wwr = wA.tile([128, DCH, DC], BF16)
                wsg = wA.tile([128, DCH, K + 2], BF16)
                wwr3 = a["W_write"].rearrange("(c p) d -> p c d", p=128)
                wsl3 = a["W_slot"].rearrange("(c p) k -> p c k", p=128)
                for cc in range(DCH):
                    nc.gpsimd.dma_start(out=wwr[:, cc, :], in_=wwr3[:, cc, :])
                    nc.gpsimd.dma_start(out=wsg[:, cc, 0:K], in_=wsl3[:, cc, :])
                nc.gpsimd.dma_start(out=wsg[:, :, K:K + 1], in_=a["W_gate"].rearrange("(c p) o -> p c o", p=128))
                nc.gpsimd.dma_start(out=wsg[:, :, K + 1:K + 2], in_=a["W_gate"].rearrange("(c p) o -> p c o", p=128))
                # cache for phases B/C/F (casting DMA fp32 -> bf16)
                nc.gpsimd.dma_start(out=cache_sb, in_=cache3)
                # attention weights (fp8 casting DMAs)
                wq3 = a["Wq"].rearrange("(c p) d -> p c d", p=128)
                wk3 = a["Wk"].rearrange("(c p) d -> p c d", p=128)
                wv3 = a["Wv"].rearrange("(c p) d -> p c d", p=128)
                wo3 = a["Wo"].rearrange("(c p) d -> p c d", p=128)
                nc.gpsimd.dma_start(out=wq8, in_=wq3)
                nc.gpsimd.dma_start(out=wk8, in_=wk3)
                nc.gpsimd.dma_start(out=wv8, in_=wv3)
                nc.gpsimd.dma_start(out=wo8, in_=wo3)

                # prologue: first two y tiles DMA'd + cast (DVE) before the
                # Ln prepass so the PE transposes start immediately
                y_bf_pre = {}
                for i in range(2):
                    y_t = pA.tile([128, D], F32, tag="y", name=f"ypre{i}")
                    nc.sync.dma_start(out=y_t, in_=y3[:, i, :])
                    y_bf = pA.tile([128, D], BF16, tag="ybf", name=f"ybfpre{i}")
                    nc.vector.tensor_copy(out=y_bf, in_=y_t)
                    y_bf_pre[i] = y_bf

                # gumbel Ln prepass: all Ln ops batched (one ACT table set)
                lnz_all = wA.tile([128, ST, K], F16)
                for g in range(8):
                    gum = pA.tile([128, 2, K], F32, tag="gum")
                    nc.sync.dma_start(out=gum, in_=gum3[:, 2 * g:2 * g + 2, :])
                    lnu = pA.tile([128, 2, K], F32, tag="lnu")
                    nc.scalar.activation(lnu, gum, AF.Ln, bias=eps8_t)
                    nc.scalar.activation(lnz_all[:, 2 * g:2 * g + 2, :], lnu, AF.Ln,
                                         bias=eps8_t, scale=-1.0)

                # persistent scatter accumulators:
                #   updates: one bank per kc (512 fp32)
                #   mass: single shared bank [128, 4] (cols 2kc:2kc+2)
                ps_upd = [psU.tile([128, DC], F32, name=f"upd{kc}", tag=f"upd{kc}")
                          for kc in range(2)]
                ps_mass = psU.tile([128, 4], F32, name="mass", tag="mass")

                prev = [None]

                def flush_scatter():
                    if prev[0] is None:
                        return
                    j, w_j, wv_j = prev[0]
                    for kc in range(2):
                        lhs = w_j[:, kc * 128:(kc + 1) * 128]
                        nc.tensor.matmul(ps_upd[kc], lhs, wv_j,
                                         start=(j == 0), stop=(j == ST - 1))
                        nc.tensor.matmul(ps_mass[:, 2 * kc:2 * kc + 2], lhs,
                                         ones_col2_bf,
                                         start=(j == 0 and kc == 0),
                                         stop=(j == ST - 1 and kc == 1),
                                         skip_group_check=True)
                    prev[0] = None

                for i in range(ST):
                    if i in y_bf_pre:
                        y_bf = y_bf_pre[i]
                    else:
                        y_t = pA.tile([128, D], F32, tag="y")
                        nc.sync.dma_start(out=y_t, in_=y3[:, i, :])
                        y_bf = pA.tile([128, D], BF16, tag="ybf")
                        nc.scalar.copy(out=y_bf, in_=y_t)

                    # transpose y tile -> yT (bf16 transposes, cheap LDW)
                    yT = pA.tile([128, D], BF16, tag="yT")
                    for g in range(2):
                        tr = psT.tile([128, 512], BF16, tag="tr")
                        for cc in range(4):
                            c = 4 * g + cc
                            nc.tensor.transpose(
                                tr[:, cc * 128:(cc + 1) * 128],
                                y_bf[:, c * 128:(c + 1) * 128],
                                ident_bf,
                            )
                        nc.vector.tensor_copy(out=yT[:, g * 512:(g + 1) * 512],
                                              in_=tr)

                    # write_vals / (logits, gate) matmuls
                    ps_wv = psWV.tile([128, DC], F32, tag="wv")
                    for c in range(DCH):
                        nc.tensor.matmul(
                            ps_wv, yT[:, c * 128:(c + 1) * 128], wwr[:, c, :],
                            start=(c == 0), stop=(c == DCH - 1),
                        )
                    ps_lg = psLG.tile([128, K + 2], F32, tag="lg")
                    for c in range(DCH):
                        nc.tensor.matmul(
                            ps_lg, yT[:, c * 128:(c + 1) * 128], wsg[:, c, :],
                            start=(c == 0), stop=False,
                        )
                    nc.tensor.matmul(ps_lg, ones_row_bf, bsg_row,
                                     start=False, stop=True)

                    # scatter matmuls for the previous tile (keeps PE dense
                    # while this tile's DVE/ACT chain runs)
                    flush_scatter()

                    # t = gamma*logits - lnz
                    t_sb = pAs.tile([128, K], F32, tag="tsb")
                    nc.vector.scalar_tensor_tensor(
                        out=t_sb, in0=ps_lg[:, 0:K], scalar=gamma_t,
                        in1=lnz_all[:, i, :], op0=ALU.mult, op1=ALU.subtract,
                    )
                    # scores = sigmoid(gate)
                    sc_e = pAs.tile([128, 1], F32, tag="sce")
                    nc.scalar.activation(sc_e, ps_lg[:, K:K + 1], AF.Exp, scale=-1.0)
                    sc1 = pAs.tile([128, 1], F32, tag="sc1")
                    nc.vector.tensor_scalar_add(sc1, sc_e, 1.0)
                    scores = pAs.tile([128, 1], F32, tag="scores")
                    nc.vector.reciprocal(scores, sc1)
                    # p_unnorm = exp(t) with fused row-sum
                    p_un = pAs.tile([128, K], F32, tag="pun")
                    rs = pAs.tile([128, 1], F32, tag="rs")
                    nc.scalar.activation(p_un, t_sb, AF.Exp, accum_out=rs)
                    rrs = pAs.tile([128, 1], F32, tag="rrs")
                    nc.vector.reciprocal(rrs, rs)
                    s2 = pAs.tile([128, 1], F32, tag="s2")
                    nc.vector.tensor_tensor(s2, scores, rrs, ALU.mult)
                    w_sb = pAs.tile([128, K], BF16, tag="wsb")
                    nc.vector.tensor_scalar_mul(w_sb, p_un, s2)
                    # wv_sb = write_vals + b_write
                    wv_sb = pAs.tile([128, DC], BF16, tag="wvsb")
                    nc.vector.scalar_tensor_tensor(
                        out=wv_sb, in0=ps_wv, scalar=1.0,
                        in1=bwr_bc, op0=ALU.mult, op1=ALU.add,
                    )
                    prev[0] = (i, w_sb, wv_sb)

                flush_scatter()

                # ------- phase B: slot update, overwrite cache rows -------
                base_t = LAYER_IDX * K // 128  # n-tile 6
                for kc in range(2):
                    mass = pAs.tile([128, 1], F32, tag="mass")
                    nc.vector.tensor_copy(out=mass, in_=ps_mass[:, 2 * kc:2 * kc + 1])
                    m1 = pAs.tile([128, 1], F32, tag="m1")
                    nc.vector.tensor_scalar_add(m1, mass, EPS)
                    rm = pAs.tile([128, 1], F32, tag="rm")
                    nc.vector.reciprocal(rm, m1)
                    m2 = pAs.tile([128, 1], F32, tag="m2")
                    nc.vector.tensor_scalar_add(m2, mass, 1.0)
                    rg = pAs.tile([128, 1], F32, tag="rg")
                    nc.vector.reciprocal(rg, m2)
                    g_t = pAs.tile([128, 1], F32, tag="gt")
                    nc.vector.tensor_tensor(g_t, mass, rg, ALU.mult)
                    co = pAs.tile([128, 1], F32, tag="co")
                    nc.vector.tensor_scalar(co, g_t, -DECAY, DECAY, ALU.mult, ALU.add)
                    cn = pAs.tile([128, 1], F32, tag="cn")
                    nc.vector.tensor_tensor(cn, g_t, rm, ALU.mult)

                    told = pA.tile([128, DC], F32, tag="told")
                    nc.vector.tensor_scalar_mul(told, cache_f[:, base_t + kc, :], co)
                    nc.vector.scalar_tensor_tensor(
                        out=cache_sb[:, base_t + kc, :],
                        in0=ps_upd[kc], scalar=cn, in1=told,
                        op0=ALU.mult, op1=ALU.add,
                    )

            # ============== phases C + D + E + F (shared PSUM) =============
            # one PSUM layout: 2x [128,1024] shared slots (C-transposes,
            # D-projections, E attention scores, F o-proj) + ao 2 + den 1
            # + 1 spare slot bank, so the scheduler can overlap phases.
            with (
                tc.tile_pool(name="psS", bufs=2, space="PSUM") as psS,
                tc.tile_pool(name="pEs", bufs=2) as pEs,
                tc.tile_pool(name="pF", bufs=2) as pF,
                tc.tile_pool(name="psX", bufs=2, space="PSUM") as psX,
                tc.tile_pool(name="psAo", bufs=1, space="PSUM") as psAo,
            ):
                # ------- phase C: cache2 -> cache2T (fp8) ------------------
                evac_flip = [0]

                def evac_copy(out_ap, in_ap):
                    if evac_flip[0] % 2 == 0:
                        nc.scalar.copy(out=out_ap, in_=in_ap)
                    else:
                        nc.vector.tensor_copy(out=out_ap, in_=in_ap)
                    evac_flip[0] += 1

                def d_slot(idx, name):
                    if idx % 2 == 0:
                        return psS.tile([128, 512], F32, tag="att", name=name)
                    return psX.tile([128, 512], F32, tag="x", name=name)

                slot_i = [0]
                for j in range(4):
                    for tg in range(4):
                        ps = (psS.tile([128, 512], BF16, tag="att", name=f"ctr{j}_{tg}")
                              if tg % 2 == 0 else
                              psX.tile([128, 512], BF16, tag="x", name=f"ctr{j}_{tg}"))
                        for tt in range(4):
                            t = tg * 4 + tt
                            nc.tensor.transpose(
                                ps[:, tt * 128:(tt + 1) * 128],
                                cache_sb[:, t, j * 128:(j + 1) * 128],
                                ident_bf,
                            )
                        evac_copy(c2t[:, j, tg * 512:(tg + 1) * 512], ps)

                # ------- phase D: projections (fp8 DoubleRow) --------------
                # v first (needed early in E), then k/q per n-chunk so the
                # attention exp stream can begin before q/k fully finish
                for m in range(NT):
                    ps = d_slot(slot_i[0], f"v{m}"); slot_i[0] += 1
                    for g in range(2):
                        nc.tensor.matmul(
                            ps, c2t[:, 2 * g:2 * g + 2, m * 128:(m + 1) * 128],
                            wv8[:, 2 * g:2 * g + 2, :],
                            start=(g == 0), stop=False, perf_mode=DR,
                        )
                    nc.tensor.matmul(ps, ones_row_bf, bvr_row,
                                     start=False, stop=True)
                    evac_copy(v_sb[:, m, :], ps)
                for c in range(4):
                    for dst, w8, b_col in ((kT, wk8, bk_col), (qT, wq8, bq_col)):
                        for h in range(H):
                            ps = d_slot(slot_i[0], f"qk{c}_{h}_{0 if dst is kT else 1}")
                            slot_i[0] += 1
                            for g in range(2):
                                nc.tensor.matmul(
                                    ps, w8[:, 2 * g:2 * g + 2, h * 128:(h + 1) * 128],
                                    c2t[:, 2 * g:2 * g + 2, c * 512:(c + 1) * 512],
                                    start=(g == 0), stop=(g == 1), perf_mode=DR,
                                )
                            if evac_flip[0] % 2 == 0:
                                nc.scalar.activation(
                                    dst[:, h, c * 512:(c + 1) * 512], ps,
                                    AF.Identity, bias=b_col[:, h:h + 1])
                            else:
                                nc.vector.tensor_scalar_add(
                                    dst[:, h, c * 512:(c + 1) * 512], ps,
                                    b_col[:, h:h + 1])
                            evac_flip[0] += 1
                def emit_F_half(c, half, st):
                    # phase F o-proj + LN stats for n-tile 2c+half
                    if half == 0:
                        st["mean2"] = pF.tile([128, 2], F32, tag="mean2",
                                              name=f"mean2_{c}")
                        st["var2"] = pF.tile([128, 2], F32, tag="var2",
                                             name=f"var2_{c}")
                        st["r"] = []
                    mean2, var2 = st["mean2"], st["var2"]
                    t = 2 * c + half
                    ps_o = psS.tile([128, DC], F32, tag="att",
                                    name=f"o{c}_{half}")
                    for g in range(2):
                        nc.tensor.matmul(
                            ps_o, aoT[:, 2 * g:2 * g + 2, t * 128:(t + 1) * 128],
                            wo8[:, 2 * g:2 * g + 2, :],
                            start=(g == 0), stop=False, perf_mode=DR,
                        )
                    nc.tensor.matmul(ps_o, ones_row_bf, bor_row,
                                     start=False, stop=True)
                    r_t = pF.tile([128, DC], F32, tag="r")
                    rsum = pF.tile([128, 1], F32, tag="rsum")
                    nc.vector.scalar_tensor_tensor(
                        out=r_t, in0=ps_o, scalar=1.0,
                        in1=cache_f[:, t, :],
                        op0=ALU.mult, op1=ALU.add, accum_out=rsum,
                    )
                    nc.vector.tensor_scalar_mul(
                        mean2[:, half:half + 1], rsum, 1.0 / DC)
                    scratch = pF.tile([128, DC], F32, tag="scratch")
                    nc.vector.scalar_tensor_tensor(
                        out=scratch, in0=r_t,
                        scalar=mean2[:, half:half + 1], in1=r_t,
                        op0=ALU.subtract, op1=ALU.mult,
                        accum_out=var2[:, half:half + 1],
                    )
                    st["r"].append(r_t)

                def emit_F_tail(c, st):
                    mean2, var2, r_ts = st["mean2"], st["var2"], st["r"]
                    # rstd = 1/sqrt(var/DC + 1e-5), quake + 2 Newton (DVE only)
                    nc.vector.tensor_scalar(var2, var2, 1.0 / DC, 1e-5,
                                            ALU.mult, ALU.add)
                    vh = pF.tile([128, 2], F32, tag="vh")
                    nc.vector.tensor_scalar_mul(vh, var2, -0.5)
                    qi = pF.tile([128, 2], I32, tag="qi")
                    nc.vector.tensor_scalar(qi, var2.bitcast(I32), 1, -1,
                                            ALU.logical_shift_right, ALU.bitwise_xor)
                    rstd = pF.tile([128, 2], F32, tag="rstd")
                    nc.vector.tensor_scalar_add(rstd.bitcast(I32), qi, QMAGIC)
                    yy = pF.tile([128, 2], F32, tag="yy")
                    for _ in range(2):
                        nc.vector.tensor_tensor(yy, rstd, rstd, ALU.mult)
                        nc.vector.tensor_tensor(yy, yy, vh, ALU.mult)
                        nc.vector.tensor_scalar_add(yy, yy, 1.5)
                        nc.vector.tensor_tensor(rstd, rstd, yy, ALU.mult)
                    for half in range(2):
                        t = 2 * c + half
                        t1 = pF.tile([128, DC], F32, tag="t1")
                        nc.vector.tensor_scalar(
                            t1, r_ts[half], mean2[:, half:half + 1],
                            rstd[:, half:half + 1], ALU.subtract, ALU.mult)
                        t2 = pF.tile([128, DC], F32, tag="t2")
                        nc.vector.scalar_tensor_tensor(
                            out=t2, in0=t1, scalar=1.0, in1=lng_bc,
                            op0=ALU.mult, op1=ALU.mult,
                        )
                        o_sb = pF.tile([128, DC], F32, tag="osb")
                        nc.vector.scalar_tensor_tensor(
                            out=o_sb, in0=t2, scalar=1.0, in1=lnb_bc,
                            op0=ALU.mult, op1=ALU.add,
                        )
                        nc.sync.dma_start(out=out3[:, t, :], in_=o_sb)

                f_state = {}
                for c in range(NCH):
                    # QK/exp stream with PV (fp8 DoubleRow) + denominator
                    # matmuls interleaved once the previous chunk's ao banks
                    # have drained (pairs 0..6 after QK m=9..15, pair 7 last)
                    ps_aoA = psAo.tile([128, 512], F32, name=f"aoA{c}", tag="aoA")
                    ps_aoB = psAo.tile([128, 512], F32, name=f"aoB{c}", tag="aoB")
                    ps_denA = psX.tile([128, 512], F32, name=f"denA{c}", tag="x")
                    ps_denB = psX.tile([128, 512], F32, name=f"denB{c}", tag="x")

                    def emit_pair(j):
                        for h in range(H):
                            ps_ao = ps_aoA if h < 2 else ps_aoB
                            nc.tensor.matmul(
                                ps_ao[:, (h % 2) * CL:(h % 2 + 1) * CL],
                                v_sb[:, 2 * j:2 * j + 2, h * 128:(h + 1) * 128],
                                pT[:, h, 2 * j:2 * j + 2, :],
                                start=(j == 0 and h % 2 == 0), stop=False,
                                perf_mode=DR,
                            )
                        # denominators: DoubleRow column-sums, h-pair per bank
                        for h in range(H):
                            ps_den = ps_denA if h < 2 else ps_denB
                            nc.tensor.matmul(
                                ps_den[0:1, (h % 2) * CL:(h % 2 + 1) * CL],
                                ones8p[:, :, 0:1],
                                pT[:, h, 2 * j:2 * j + 2, :],
                                start=(j == 0), stop=False,
                                perf_mode=DR,
                                skip_group_check=True,
                            )

                    def emit_single(m2):
                        # last two m-tiles as plain fp8 matmuls so m14's work
                        # overlaps exp(m15) and the boundary tail stays short
                        last = (m2 == NT - 1)
                        for h in range(H):
                            ps_ao = ps_aoA if h < 2 else ps_aoB
                            nc.tensor.matmul(
                                ps_ao[:, (h % 2) * CL:(h % 2 + 1) * CL],
                                v_sb[:, m2, h * 128:(h + 1) * 128],
                                pT[:, h, m2, :],
                                start=False, stop=(last and h % 2 == 1),
                            )
                        for h in range(H):
                            ps_den = ps_denA if h < 2 else ps_denB
                            nc.tensor.matmul(
                                ps_den[0:1, (h % 2) * CL:(h % 2 + 1) * CL],
                                ones8p[:, 0, 0:1],
                                pT[:, h, m2, :],
                                start=False, stop=last,
                                skip_group_check=True,
                            )

                    for m in range(NT):
                        ps_a = psS.tile([128, H * CL], F32, tag="att")
                        for h in range(H):
                            nc.tensor.matmul(
                                ps_a[:, h * CL:(h + 1) * CL],
                                kT[:, h, m * 128:(m + 1) * 128],
                                qT[:, h, c * CL:(c + 1) * CL],
                                start=True, stop=True,
                            )
                        nc.scalar.activation(
                            pT[:, :, m, :],
                            ps_a.rearrange("p (h n) -> p h n", h=H),
                            AF.Exp, scale=ATT_SCALE)
                        if c > 0:
                            if m == 7:
                                emit_F_half(c - 1, 0, f_state)
                            elif m == 8:
                                emit_F_half(c - 1, 1, f_state)
                                emit_F_tail(c - 1, f_state)
                        if m >= 9 and m <= 15:
                            emit_pair(m - 9)
                        if m == 15:
                            emit_single(14)
                    emit_single(15)

                    # 1/den via one Newton step from y0=2^-11 (den ~ 2048),
                    # broadcast den rows, fused normalize evac
                    den_row = pEs.tile([1, H * CL], F32, tag="denrow")
                    nc.vector.tensor_copy(out=den_row[:, 0:512], in_=ps_denA[0:1, :])
                    nc.vector.tensor_copy(out=den_row[:, 512:1024], in_=ps_denB[0:1, :])
                    bc_den = pEs.tile([128, H * CL], F32, tag="bcden")
                    nc.gpsimd.partition_broadcast(bc_den, den_row)
                    # y1 = y0*(2 - den*y0),  y0 = 2^-11
                    Y0 = 1.0 / 2048.0
                    t2_nr = pEs.tile([128, H * CL], F32, tag="t2nr")
                    nc.vector.tensor_scalar(t2_nr, bc_den, -Y0, 2.0,
                                            ALU.mult, ALU.add)
                    rden_bc = pEs.tile([128, H * CL], F32, tag="rdenbc")
                    nc.vector.tensor_scalar_mul(rden_bc, t2_nr, Y0)
                    for h in range(H):
                        ps_ao = ps_aoA if h < 2 else ps_aoB
                        nc.vector.scalar_tensor_tensor(
                            out=aoT[:, h, c * CL:(c + 1) * CL],
                            in0=ps_ao[:, (h % 2) * CL:(h % 2 + 1) * CL],
                            scalar=1.0, in1=rden_bc[:, h * CL:(h + 1) * CL],
                            op0=ALU.mult, op1=ALU.mult,
                        )

                emit_F_half(NCH - 1, 0, f_state)
                emit_F_half(NCH - 1, 1, f_state)
                emit_F_tail(NCH - 1, f_state)

    nc.compile()
    return nc


_NC_CACHE = {}


def _get_nc():
    if "nc" not in _NC_CACHE:
        _NC_CACHE["nc"] = _build()
    return _NC_CACHE["nc"]


def _in_maps(inputs):
    per_batch = {"y", "cache", "gumbel_u"}
    maps = []
    for b in range(B):
        m = {}
        for name in _INPUT_SPECS:
            arr = np.ascontiguousarray(np.asarray(inputs[name], dtype=np.float32))
            m[name] = arr[b] if name in per_batch else arr
        maps.append(m)
    return maps


def _execute(inputs, trace=False):
    nc = _get_nc()
    res = run_bass_kernel_spmd(nc, _in_maps(inputs), list(range(B)), trace=trace)
    out = np.stack([res.results[b]["out"] for b in range(B)]).astype(np.float32)
    return out, res


def kernel(**inputs) -> np.ndarray:
    out, _ = _execute(inputs)
    return out


# revision 93
# speedup vs baseline: 1.2034x; 1.0301x over previous
"""DLSMN scatter-memory + cache self-attention kernel for Trainium2.

Data-parallel over batch: batch b runs on NeuronCore b (8 cores), no
collectives.  Inside one core (one batch):

  phase A: per 128-token tile of y: PE-transpose y (f32r transposes) ->
           yT (bf16), bf16 matmuls (FWL weight loads) for write_vals /
           (logits,gate), gumbel-softmax routing (all Ln batched first ->
           2 ACT table loads total), weighted-scatter matmuls into 2 PSUM
           banks + a shared mass bank (3 banks total).
  phase B: slot update  upd = (1-g)*DECAY*old + g*updates/(mass+eps).
  phase C: PE-transpose cache2 -> cache2T (fp8e4).
  phase D: q/k/v projections with fp8 DoubleRow matmuls, bias folded
           into the PSUM evacuations: qT/kT bf16, v fp8e4.
  phase E: attention transposed, QK^T bf16.  Chunk structure: all 16 QK
           tiles first (exp on ScalarE streams into a persistent fp8 pT
           buffer), then the o-projection of the PREVIOUS chunk, then PV
           (fp8 DoubleRow) + denominator matmuls.  The previous chunk's
           softmax-normalization tail overlaps this chunk's QK segment.
  phase F: pipelined one chunk behind phase E: o-projection in fp8
           DoubleRow, residual + layernorm with a DVE-only Quake rsqrt,
           output DMA per n-tile.
"""

import numpy as np

import concourse.bacc as bacc
import concourse.mybir as mybir
import concourse.tile as tile
from concourse.bass_utils import run_bass_kernel_spmd
from concourse.masks import make_identity

F32 = mybir.dt.float32
F32R = mybir.dt.float32r
F16 = mybir.dt.float16
BF16 = mybir.dt.bfloat16
FP8 = mybir.dt.float8e4
I32 = mybir.dt.int32
AF = mybir.ActivationFunctionType
ALU = mybir.AluOpType
DR = mybir.MatmulPerfMode.DoubleRow

B = 8
S = 2048
D = 1024
DC = 512
K = 256
L = 8
H = 4
HD = 128
N = L * K
LAYER_IDX = 3
DECAY = 0.9
EPS = 1e-6
ST = S // 128   # 16 token tiles
NT = N // 128   # 16 slot tiles
DCH = D // 128  # 8 d_model chunks
CL = 256        # attention n-chunk length
NCH = N // CL   # 8 attention chunks
ATT_SCALE = float(1.0 / np.sqrt(np.float32(HD)))
QMAGIC = 0x5F3759DF + 1  # quake rsqrt magic (+1 for the xor-negate trick)

_INPUT_SPECS = {
    "y": (S, D), "cache": (N, DC), "gumbel_u": (S, K),
    "W_gate": (D, 1), "b_gate": (1,), "W_slot": (D, K), "b_slot": (K,),
    "gamma": (1,), "W_write": (D, DC), "b_write": (DC,),
    "Wq": (DC, DC), "bq": (DC,), "Wk": (DC, DC), "bk": (DC,),
    "Wv": (DC, DC), "bv": (DC,), "Wo": (DC, DC), "bo": (DC,),
    "ln_g": (DC,), "ln_b": (DC,),
}


def _build():
    nc = bacc.Bacc("TRN2", target_bir_lowering=False, debug=False, num_devices=B)

    a = {
        name: nc.dram_tensor(name, list(shape), F32, kind="ExternalInput").ap()
        for name, shape in _INPUT_SPECS.items()
    }
    out_dram = nc.dram_tensor("out", [N, DC], F32, kind="ExternalOutput").ap()

    y3 = a["y"].rearrange("(t p) d -> p t d", p=128)
    gum3 = a["gumbel_u"].rearrange("(t p) k -> p t k", p=128)
    cache3 = a["cache"].rearrange("(t p) d -> p t d", p=128)
    out3 = out_dram.rearrange("(t p) d -> p t d", p=128)

    with tile.TileContext(nc) as tc:
        with (
            tc.tile_pool(name="const", bufs=1) as const,
            tc.tile_pool(name="cachep", bufs=1) as cachep,
            tc.tile_pool(name="attn", bufs=1) as attn,
        ):
            # ---------------- constants ------------------------------------
            ident = const.tile([128, 128], F32)
            make_identity(nc, ident)
            ident_bf = const.tile([128, 128], BF16)
            nc.vector.tensor_copy(out=ident_bf, in_=ident)
            ident_r = const.tile([128, 128], F32R)
            nc.vector.tensor_copy(out=ident_r, in_=ident)
            ones_row_bf = const.tile([1, 128], BF16)
            nc.vector.memset(ones_row_bf, 1.0)
            ones_col2_bf = const.tile([128, 2], BF16)
            nc.vector.memset(ones_col2_bf, 1.0)
            ones8p = const.tile([128, 2, 16], FP8)
            nc.vector.memset(ones8p, 1.0)
            eps8_t = const.tile([128, 1], F32)
            nc.vector.memset(eps8_t, 1e-8)
            gamma_t = const.tile([128, 1], F32)
            nc.sync.dma_start(out=gamma_t, in_=a["gamma"].unsqueeze(0).to_broadcast([128, 1]))
            bwr_bc = const.tile([128, DC], F32)
            nc.gpsimd.dma_start(out=bwr_bc, in_=a["b_write"].unsqueeze(0).to_broadcast([128, DC]))
            bsg_row = const.tile([1, K + 2], BF16)
            nc.gpsimd.dma_start(out=bsg_row[:, 0:K], in_=a["b_slot"].unsqueeze(0))
            nc.gpsimd.dma_start(out=bsg_row[:, K:K + 1], in_=a["b_gate"].unsqueeze(0))
            nc.gpsimd.dma_start(out=bsg_row[:, K + 1:K + 2], in_=a["b_gate"].unsqueeze(0))
            # constants needed only from phase CD/F onward are declared here
            # but DMA'd after the phase-A weights (gpsimd queue order)
            lng_bc = const.tile([128, DC], F32)
            lnb_bc = const.tile([128, DC], F32)
            bq_col = const.tile([128, H], F32)
            bk_col = const.tile([128, H], F32)
            bor_row = const.tile([1, DC], BF16)
            bvr_row = const.tile([1, DC], BF16)

            cache_sb = cachep.tile([128, NT, DC], BF16)
            cache_f = cache_sb

            # ---------------- persistent attention tiles -------------------
            c2t = attn.tile([128, 4, N], FP8)
            qT = attn.tile([128, H, N], BF16)
            kT = attn.tile([128, H, N], BF16)
            v_sb = attn.tile([128, NT, DC], FP8)
            wq8 = attn.tile([128, 4, DC], FP8)
            wk8 = attn.tile([128, 4, DC], FP8)
            wv8 = attn.tile([128, 4, DC], FP8)
            wo8 = attn.tile([128, 4, DC], FP8)
            aoT = attn.tile([128, H, N], FP8)
            pT = attn.tile([128, H, NT, CL], FP8)

            # ======================= phase A + B ===========================
            with (
                tc.tile_pool(name="wA", bufs=1) as wA,
                tc.tile_pool(name="pA", bufs=2) as pA,
                tc.tile_pool(name="pAs", bufs=3) as pAs,
                tc.tile_pool(name="psT", bufs=2, space="PSUM") as psT,
                tc.tile_pool(name="psWV", bufs=2, space="PSUM") as psWV,
                tc.tile_pool(name="psLG", bufs=1, space="PSUM") as psLG,
                tc.tile_pool(name="psU", bufs=1, space="PSUM") as psU,
            ):
                # A-phase weights: fast f32 DMA to staging on the vector
                # queue, chunked DVE casts -> bf16 (the gpsimd casting-DMA
                # path is only ~130GB/s and would gate phases A and CD)
                wwr = wA.tile([128, DCH, DC], BF16)
                wsg = wA.tile([128, DCH, K + 2], BF16)
                wwr3 = a["W_write"].rearrange("(c p) d -> p c d", p=128)
                wsl3 = a["W_slot"].rearrange("(c p) k -> p c k", p=128)
                for cc in range(DCH):
                    nc.gpsimd.dma_start(out=wwr[:, cc, :], in_=wwr3[:, cc, :])
                    nc.gpsimd.dma_start(out=wsg[:, cc, 0:K], in_=wsl3[:, cc, :])
                nc.gpsimd.dma_start(out=wsg[:, :, K:K + 1], in_=a["W_gate"].rearrange("(c p) o -> p c o", p=128))
                nc.gpsimd.dma_start(out=wsg[:, :, K + 1:K + 2], in_=a["W_gate"].rearrange("(c p) o -> p c o", p=128))
                # cache for phases B/C/F (casting DMA fp32 -> bf16)
                nc.gpsimd.dma_start(out=cache_sb, in_=cache3)
                # attention weights (fp8 casting DMAs)
                wq3 = a["Wq"].rearrange("(c p) d -> p c d", p=128)
                wk3 = a["Wk"].rearrange("(c p) d -> p c d", p=128)
                wv3 = a["Wv"].rearrange("(c p) d -> p c d", p=128)
                wo3 = a["Wo"].rearrange("(c p) d -> p c d", p=128)
                nc.gpsimd.dma_start(out=wq8, in_=wq3)
                nc.gpsimd.dma_start(out=wk8, in_=wk3)
                nc.gpsimd.dma_start(out=wv8, in_=wv3)
                nc.gpsimd.dma_start(out=wo8, in_=wo3)
                nc.gpsimd.dma_start(out=lng_bc, in_=a["ln_g"].unsqueeze(0).to_broadcast([128, DC]))
                nc.gpsimd.dma_start(out=lnb_bc, in_=a["ln_b"].unsqueeze(0).to_broadcast([128, DC]))
                nc.gpsimd.dma_start(out=bq_col, in_=a["bq"].rearrange("(h p) -> p h", p=128))
                nc.gpsimd.dma_start(out=bk_col, in_=a["bk"].rearrange("(h p) -> p h", p=128))
                nc.gpsimd.dma_start(out=bor_row, in_=a["bo"].unsqueeze(0))
                nc.gpsimd.dma_start(out=bvr_row, in_=a["bv"].unsqueeze(0))

                # prologue: first two y tiles DMA'd + cast (DVE) before the
                # Ln prepass so the PE transposes start immediately
                y_bf_pre = {}
                for i in range(2):
                    y_t = pA.tile([128, D], F32, tag="y", name=f"ypre{i}")
                    nc.sync.dma_start(out=y_t, in_=y3[:, i, :])
                    y_bf = pA.tile([128, D], BF16, tag="ybf", name=f"ybfpre{i}")
                    nc.vector.tensor_copy(out=y_bf, in_=y_t)
                    y_bf_pre[i] = y_bf

                # gumbel Ln prepass: all Ln ops batched (one ACT table set)
                lnz_all = wA.tile([128, ST, K], F16)
                for g in range(8):
                    gum = pA.tile([128, 2, K], F32, tag="gum")
                    nc.sync.dma_start(out=gum, in_=gum3[:, 2 * g:2 * g + 2, :])
                    lnu = pA.tile([128, 2, K], F32, tag="lnu")
                    nc.scalar.activation(lnu, gum, AF.Ln, bias=eps8_t)
                    nc.scalar.activation(lnz_all[:, 2 * g:2 * g + 2, :], lnu, AF.Ln,
                                         bias=eps8_t, scale=-1.0)

                # persistent scatter accumulators:
                #   updates: one bank per kc (512 fp32)
                #   mass: single shared bank [128, 4] (cols 2kc:2kc+2)
                ps_upd = [psU.tile([128, DC], F32, name=f"upd{kc}", tag=f"upd{kc}")
                          for kc in range(2)]
                ps_mass = psU.tile([128, 4], F32, name="mass", tag="mass")

                prev = [None]

                def flush_scatter():
                    if prev[0] is None:
                        return
                    j, w_j, wv_j = prev[0]
                    for kc in range(2):
                        lhs = w_j[:, kc * 128:(kc + 1) * 128]
                        nc.tensor.matmul(ps_upd[kc], lhs, wv_j,
                                         start=(j == 0), stop=(j == ST - 1))
                        nc.tensor.matmul(ps_mass[:, 2 * kc:2 * kc + 2], lhs,
                                         ones_col2_bf,
                                         start=(j == 0 and kc == 0),
                                         stop=(j == ST - 1 and kc == 1),
                                         skip_group_check=True)
                    prev[0] = None

                for i in range(ST):
                    if i in y_bf_pre:
                        y_bf = y_bf_pre[i]
                    else:
                        y_t = pA.tile([128, D], F32, tag="y")
                        nc.sync.dma_start(out=y_t, in_=y3[:, i, :])
                        y_bf = pA.tile([128, D], BF16, tag="ybf")
                        nc.scalar.copy(out=y_bf, in_=y_t)

                    # transpose y tile -> yT (bf16 transposes, cheap LDW)
                    yT = pA.tile([128, D], BF16, tag="yT")
                    for g in range(2):
                        tr = psT.tile([128, 512], BF16, tag="tr")
                        for cc in range(4):
                            c = 4 * g + cc
                            nc.tensor.transpose(
                                tr[:, cc * 128:(cc + 1) * 128],
                                y_bf[:, c * 128:(c + 1) * 128],
                                ident_bf,
                            )
                        nc.vector.tensor_copy(out=yT[:, g * 512:(g + 1) * 512],
                                              in_=tr)

                    # write_vals / (logits, gate) matmuls
                    ps_wv = psWV.tile([128, DC], F32, tag="wv")
                    for c in range(DCH):
                        nc.tensor.matmul(
                            ps_wv, yT[:, c * 128:(c + 1) * 128], wwr[:, c, :],
                            start=(c == 0), stop=(c == DCH - 1),
                        )
                    ps_lg = psLG.tile([128, K + 2], F32, tag="lg")
                    for c in range(DCH):
                        nc.tensor.matmul(
                            ps_lg, yT[:, c * 128:(c + 1) * 128], wsg[:, c, :],
                            start=(c == 0), stop=False,
                        )
                    nc.tensor.matmul(ps_lg, ones_row_bf, bsg_row,
                                     start=False, stop=True)

                    # scatter matmuls for the previous tile (keeps PE dense
                    # while this tile's DVE/ACT chain runs)
                    flush_scatter()

                    # t = gamma*logits - lnz
                    t_sb = pAs.tile([128, K], F32, tag="tsb")
                    nc.vector.scalar_tensor_tensor(
                        out=t_sb, in0=ps_lg[:, 0:K], scalar=gamma_t,
                        in1=lnz_all[:, i, :], op0=ALU.mult, op1=ALU.subtract,
                    )
                    # scores = sigmoid(gate)
                    sc_e = pAs.tile([128, 1], F32, tag="sce")
                    nc.scalar.activation(sc_e, ps_lg[:, K:K + 1], AF.Exp, scale=-1.0)
                    sc1 = pAs.tile([128, 1], F32, tag="sc1")
                    nc.vector.tensor_scalar_add(sc1, sc_e, 1.0)
                    scores = pAs.tile([128, 1], F32, tag="scores")
                    nc.vector.reciprocal(scores, sc1)
                    # p_unnorm = exp(t) with fused row-sum
                    p_un = pAs.tile([128, K], F32, tag="pun")
                    rs = pAs.tile([128, 1], F32, tag="rs")
                    nc.scalar.activation(p_un, t_sb, AF.Exp, accum_out=rs)
                    rrs = pAs.tile([128, 1], F32, tag="rrs")
                    nc.vector.reciprocal(rrs, rs)
                    s2 = pAs.tile([128, 1], F32, tag="s2")
                    nc.vector.tensor_tensor(s2, scores, rrs, ALU.mult)
                    w_sb = pAs.tile([128, K], BF16, tag="wsb")
                    nc.vector.tensor_scalar_mul(w_sb, p_un, s2)
                    # wv_sb = write_vals + b_write
                    wv_sb = pAs.tile([128, DC], BF16, tag="wvsb")
                    nc.vector.scalar_tensor_tensor(
                        out=wv_sb, in0=ps_wv, scalar=1.0,
                        in1=bwr_bc, op0=ALU.mult, op1=ALU.add,
                    )
                    prev[0] = (i, w_sb, wv_sb)

                flush_scatter()

                # ------- phase B: slot update, overwrite cache rows -------
                base_t = LAYER_IDX * K // 128  # n-tile 6
                for kc in range(2):
                    mass = pAs.tile([128, 1], F32, tag="mass")
                    nc.vector.tensor_copy(out=mass, in_=ps_mass[:, 2 * kc:2 * kc + 1])
                    m1 = pAs.tile([128, 1], F32, tag="m1")
                    nc.vector.tensor_scalar_add(m1, mass, EPS)
                    rm = pAs.tile([128, 1], F32, tag="rm")
                    nc.vector.reciprocal(rm, m1)
                    m2 = pAs.tile([128, 1], F32, tag="m2")
                    nc.vector.tensor_scalar_add(m2, mass, 1.0)
                    rg = pAs.tile([128, 1], F32, tag="rg")
                    nc.vector.reciprocal(rg, m2)
                    g_t = pAs.tile([128, 1], F32, tag="gt")
                    nc.vector.tensor_tensor(g_t, mass, rg, ALU.mult)
                    co = pAs.tile([128, 1], F32, tag="co")
                    nc.vector.tensor_scalar(co, g_t, -DECAY, DECAY, ALU.mult, ALU.add)
                    cn = pAs.tile([128, 1], F32, tag="cn")
                    nc.vector.tensor_tensor(cn, g_t, rm, ALU.mult)

                    told = pA.tile([128, DC], F32, tag="told")
                    nc.vector.tensor_scalar_mul(told, cache_f[:, base_t + kc, :], co)
                    nc.vector.scalar_tensor_tensor(
                        out=cache_sb[:, base_t + kc, :],
                        in0=ps_upd[kc], scalar=cn, in1=told,
                        op0=ALU.mult, op1=ALU.add,
                    )

            # ============== phases C + D + E + F (shared PSUM) =============
            # one PSUM layout: 2x [128,1024] shared slots (C-transposes,
            # D-projections, E attention scores, F o-proj) + ao 2 + den 1
            # + 1 spare slot bank, so the scheduler can overlap phases.
            with (
                tc.tile_pool(name="psS", bufs=2, space="PSUM") as psS,
                tc.tile_pool(name="pEs", bufs=2) as pEs,
                tc.tile_pool(name="pF", bufs=2) as pF,
                tc.tile_pool(name="psX", bufs=2, space="PSUM") as psX,
                tc.tile_pool(name="psAo", bufs=1, space="PSUM") as psAo,
            ):
                # ------- phase C: cache2 -> cache2T (fp8) ------------------
                evac_flip = [0]

                def evac_copy(out_ap, in_ap):
                    if evac_flip[0] % 2 == 0:
                        nc.scalar.copy(out=out_ap, in_=in_ap)
                    else:
                        nc.vector.tensor_copy(out=out_ap, in_=in_ap)
                    evac_flip[0] += 1

                def d_slot(idx, name):
                    if idx % 2 == 0:
                        return psS.tile([128, 512], F32, tag="att", name=name)
                    return psX.tile([128, 512], F32, tag="x", name=name)

                slot_i = [0]
                for j in range(4):
                    for tg in range(4):
                        ps = (psS.tile([128, 512], BF16, tag="att", name=f"ctr{j}_{tg}")
                              if tg % 2 == 0 else
                              psX.tile([128, 512], BF16, tag="x", name=f"ctr{j}_{tg}"))
                        for tt in range(4):
                            t = tg * 4 + tt
                            nc.tensor.transpose(
                                ps[:, tt * 128:(tt + 1) * 128],
                                cache_sb[:, t, j * 128:(j + 1) * 128],
                                ident_bf,
                            )
                        evac_copy(c2t[:, j, tg * 512:(tg + 1) * 512], ps)

                # ------- phase D: projections (fp8 DoubleRow) --------------
                # v first (needed early in E), then k/q per n-chunk so the
                # attention exp stream can begin before q/k fully finish
                for m in range(NT):
                    ps = d_slot(slot_i[0], f"v{m}"); slot_i[0] += 1
                    for g in range(2):
                        nc.tensor.matmul(
                            ps, c2t[:, 2 * g:2 * g + 2, m * 128:(m + 1) * 128],
                            wv8[:, 2 * g:2 * g + 2, :],
                            start=(g == 0), stop=False, perf_mode=DR,
                        )
                    nc.tensor.matmul(ps, ones_row_bf, bvr_row,
                                     start=False, stop=True)
                    evac_copy(v_sb[:, m, :], ps)
                for c in range(4):
                    for dst, w8, b_col in ((kT, wk8, bk_col), (qT, wq8, bq_col)):
                        for h in range(H):
                            ps = d_slot(slot_i[0], f"qk{c}_{h}_{0 if dst is kT else 1}")
                            slot_i[0] += 1
                            for g in range(2):
                                nc.tensor.matmul(
                                    ps, w8[:, 2 * g:2 * g + 2, h * 128:(h + 1) * 128],
                                    c2t[:, 2 * g:2 * g + 2, c * 512:(c + 1) * 512],
                                    start=(g == 0), stop=(g == 1), perf_mode=DR,
                                )
                            if evac_flip[0] % 2 == 0:
                                nc.scalar.activation(
                                    dst[:, h, c * 512:(c + 1) * 512], ps,
                                    AF.Identity, bias=b_col[:, h:h + 1])
                            else:
                                nc.vector.tensor_scalar_add(
                                    dst[:, h, c * 512:(c + 1) * 512], ps,
                                    b_col[:, h:h + 1])
                            evac_flip[0] += 1
                def emit_F_half(c, half, st):
                    # phase F o-proj + LN stats for n-tile 2c+half
                    if half == 0:
                        st["mean2"] = pF.tile([128, 2], F32, tag="mean2",
                                              name=f"mean2_{c}")
                        st["var2"] = pF.tile([128, 2], F32, tag="var2",
                                             name=f"var2_{c}")
                        st["r"] = []
                    mean2, var2 = st["mean2"], st["var2"]
                    t = 2 * c + half
                    ps_o = psS.tile([128, DC], F32, tag="att",
                                    name=f"o{c}_{half}")
                    for g in range(2):
                        nc.tensor.matmul(
                            ps_o, aoT[:, 2 * g:2 * g + 2, t * 128:(t + 1) * 128],
                            wo8[:, 2 * g:2 * g + 2, :],
                            start=(g == 0), stop=False, perf_mode=DR,
                        )
                    nc.tensor.matmul(ps_o, ones_row_bf, bor_row,
                                     start=False, stop=True)
                    r_t = pF.tile([128, DC], F32, tag="r")
                    rsum = pF.tile([128, 1], F32, tag="rsum")
                    nc.vector.scalar_tensor_tensor(
                        out=r_t, in0=ps_o, scalar=1.0,
                        in1=cache_f[:, t, :],
                        op0=ALU.mult, op1=ALU.add, accum_out=rsum,
                    )
                    nc.vector.tensor_scalar_mul(
                        mean2[:, half:half + 1], rsum, 1.0 / DC)
                    scratch = pF.tile([128, DC], F32, tag="scratch")
                    nc.vector.scalar_tensor_tensor(
                        out=scratch, in0=r_t,
                        scalar=mean2[:, half:half + 1], in1=r_t,
                        op0=ALU.subtract, op1=ALU.mult,
                        accum_out=var2[:, half:half + 1],
                    )
                    st["r"].append(r_t)

                def emit_F_tail(c, st):
                    mean2, var2, r_ts = st["mean2"], st["var2"], st["r"]
                    # rstd = 1/sqrt(var/DC + 1e-5), quake + 2 Newton (DVE only)
                    nc.vector.tensor_scalar(var2, var2, 1.0 / DC, 1e-5,
                                            ALU.mult, ALU.add)
                    vh = pF.tile([128, 2], F32, tag="vh")
                    nc.vector.tensor_scalar_mul(vh, var2, -0.5)
                    qi = pF.tile([128, 2], I32, tag="qi")
                    nc.vector.tensor_scalar(qi, var2.bitcast(I32), 1, -1,
                                            ALU.logical_shift_right, ALU.bitwise_xor)
                    rstd = pF.tile([128, 2], F32, tag="rstd")
                    nc.vector.tensor_scalar_add(rstd.bitcast(I32), qi, QMAGIC)
                    yy = pF.tile([128, 2], F32, tag="yy")
                    for _ in range(2):
                        nc.vector.tensor_tensor(yy, rstd, rstd, ALU.mult)
                        nc.vector.tensor_tensor(yy, yy, vh, ALU.mult)
                        nc.vector.tensor_scalar_add(yy, yy, 1.5)
                        nc.vector.tensor_tensor(rstd, rstd, yy, ALU.mult)
                    for half in range(2):
                        t = 2 * c + half
                        t1 = pF.tile([128, DC], F32, tag="t1")
                        nc.vector.tensor_scalar(
                            t1, r_ts[half], mean2[:, half:half + 1],
                            rstd[:, half:half + 1], ALU.subtract, ALU.mult)
                        t2 = pF.tile([128, DC], F32, tag="t2")
                        nc.vector.scalar_tensor_tensor(
                            out=t2, in0=t1, scalar=1.0, in1=lng_bc,
                            op0=ALU.mult, op1=ALU.mult,
                        )
                        o_sb = pF.tile([128, DC], F32, tag="osb")
                        nc.vector.scalar_tensor_tensor(
                            out=o_sb, in0=t2, scalar=1.0, in1=lnb_bc,
                            op0=ALU.mult, op1=ALU.add,
                        )
                        nc.sync.dma_start(out=out3[:, t, :], in_=o_sb)

                f_state = {}
                prev_ch = None

                def emit_single_for(st, m2):
                    # last two m-tiles as plain fp8 matmuls, pipelined into
                    # the next chunk's QK stream (boundary tail removal)
                    last = (m2 == NT - 1)
                    for h in range(H):
                        ps_ao = st["aoA"] if h < 2 else st["aoB"]
                        nc.tensor.matmul(
                            ps_ao[:, (h % 2) * CL:(h % 2 + 1) * CL],
                            v_sb[:, m2, h * 128:(h + 1) * 128],
                            pT[:, h, m2, :],
                            start=False, stop=(last and h % 2 == 1),
                        )
                    for h in range(H):
                        ps_den = st["denA"] if h < 2 else st["denB"]
                        nc.tensor.matmul(
                            ps_den[0:1, (h % 2) * CL:(h % 2 + 1) * CL],
                            ones8p[:, 0, 0:1],
                            pT[:, h, m2, :],
                            start=False, stop=last,
                            skip_group_check=True,
                        )

                def finish_chunk(st):
                    # 1/den via one Newton step from y0=2^-11 (den ~ 2048),
                    # broadcast den rows, fused normalize evac
                    cc = st["c"]
                    den_row = pEs.tile([1, H * CL], F32, tag="denrow",
                                       name=f"denrow{cc}")
                    nc.vector.tensor_copy(out=den_row[:, 0:512],
                                          in_=st["denA"][0:1, :])
                    nc.vector.tensor_copy(out=den_row[:, 512:1024],
                                          in_=st["denB"][0:1, :])
                    bc_den = pEs.tile([128, H * CL], F32, tag="bcden",
                                      name=f"bcden{cc}")
                    nc.gpsimd.partition_broadcast(bc_den, den_row)
                    Y0 = 1.0 / 2048.0
                    t2_nr = pEs.tile([128, H * CL], F32, tag="t2nr",
                                     name=f"t2nr{cc}")
                    nc.vector.tensor_scalar(t2_nr, bc_den, -Y0, 2.0,
                                            ALU.mult, ALU.add)
                    rden_bc = pEs.tile([128, H * CL], F32, tag="rdenbc",
                                       name=f"rdenbc{cc}")
                    nc.vector.tensor_scalar_mul(rden_bc, t2_nr, Y0)
                    for h in range(H):
                        ps_ao = st["aoA"] if h < 2 else st["aoB"]
                        nc.vector.scalar_tensor_tensor(
                            out=aoT[:, h, cc * CL:(cc + 1) * CL],
                            in0=ps_ao[:, (h % 2) * CL:(h % 2 + 1) * CL],
                            scalar=1.0, in1=rden_bc[:, h * CL:(h + 1) * CL],
                            op0=ALU.mult, op1=ALU.mult,
                        )

                for c in range(NCH):
                    # QK/exp stream with PV (fp8 DoubleRow) + denominator
                    # matmuls interleaved once the previous chunk's ao banks
                    # have drained (pairs 0..6 after QK m=9..15, pair 7 last)
                    ps_aoA = psAo.tile([128, 512], F32, name=f"aoA{c}", tag="aoA")
                    ps_aoB = psAo.tile([128, 512], F32, name=f"aoB{c}", tag="aoB")
                    ps_denA = psX.tile([128, 512], F32, name=f"denA{c}", tag="x")
                    ps_denB = psX.tile([128, 512], F32, name=f"denB{c}", tag="x")

                    def emit_pair(j):
                        for h in range(H):
                            ps_ao = ps_aoA if h < 2 else ps_aoB
                            nc.tensor.matmul(
                                ps_ao[:, (h % 2) * CL:(h % 2 + 1) * CL],
                                v_sb[:, 2 * j:2 * j + 2, h * 128:(h + 1) * 128],
                                pT[:, h, 2 * j:2 * j + 2, :],
                                start=(j == 0 and h % 2 == 0), stop=False,
                                perf_mode=DR,
                            )
                        # denominators: DoubleRow column-sums, h-pair per bank
                        for h in range(H):
                            ps_den = ps_denA if h < 2 else ps_denB
                            nc.tensor.matmul(
                                ps_den[0:1, (h % 2) * CL:(h % 2 + 1) * CL],
                                ones8p[:, :, 0:1],
                                pT[:, h, 2 * j:2 * j + 2, :],
                                start=(j == 0), stop=False,
                                perf_mode=DR,
                                skip_group_check=True,
                            )

                    cur = {"c": c, "aoA": ps_aoA, "aoB": ps_aoB,
                           "denA": ps_denA, "denB": ps_denB}

                    for m in range(NT):
                        ps_a = psS.tile([128, H * CL], F32, tag="att")
                        for h in range(H):
                            nc.tensor.matmul(
                                ps_a[:, h * CL:(h + 1) * CL],
                                kT[:, h, m * 128:(m + 1) * 128],
                                qT[:, h, c * CL:(c + 1) * CL],
                                start=True, stop=True,
                            )
                        nc.scalar.activation(
                            pT[:, :, m, :],
                            ps_a.rearrange("p (h n) -> p h n", h=H),
                            AF.Exp, scale=ATT_SCALE)
                        if prev_ch is not None:
                            if m == 0:
                                emit_single_for(prev_ch, 14)
                            elif m == 1:
                                emit_single_for(prev_ch, 15)
                                finish_chunk(prev_ch)
                            elif m == 7:
                                emit_F_half(c - 1, 0, f_state)
                            elif m == 8:
                                emit_F_half(c - 1, 1, f_state)
                                emit_F_tail(c - 1, f_state)
                        if m >= 9 and m <= 15:
                            emit_pair(m - 9)
                    prev_ch = cur

                emit_single_for(prev_ch, 14)
                emit_single_for(prev_ch, 15)
                finish_chunk(prev_ch)
                emit_F_half(NCH - 1, 0, f_state)
                emit_F_half(NCH - 1, 1, f_state)
                emit_F_tail(NCH - 1, f_state)

    nc.compile()
    return nc


_NC_CACHE = {}


def _get_nc():
    if "nc" not in _NC_CACHE:
        _NC_CACHE["nc"] = _build()
    return _NC_CACHE["nc"]


def _in_maps(inputs):
    per_batch = {"y", "cache", "gumbel_u"}
    maps = []
    for b in range(B):
        m = {}
        for name in _INPUT_SPECS:
            arr = np.ascontiguousarray(np.asarray(inputs[name], dtype=np.float32))
            m[name] = arr[b] if name in per_batch else arr
        maps.append(m)
    return maps


def _execute(inputs, trace=False):
    nc = _get_nc()
    res = run_bass_kernel_spmd(nc, _in_maps(inputs), list(range(B)), trace=trace)
    out = np.stack([res.results[b]["out"] for b in range(B)]).astype(np.float32)
    return out, res


def kernel(**inputs) -> np.ndarray:
    out, _ = _execute(inputs)
    return out
